# revision 71
# baseline (speedup 1.0000x reference)
"""Grouped-query attention kernel for 8 Trainium2 NeuronCores.

Problem (hardcoded): x [2, 512, 16, 16, 16] f32, Wq/Wk/Wv/Wo [512, 512],
biases [512]. G=4 heads of dim 128, N=4096 tokens. out = x + Wo@attn.

Sharding: one (batch, group) pair per core -> 8 cores, no cross-core
communication. Each core computes its group's Q/K/V projections, the
full 4096x4096 attention for its (b, g), and a partial output
projection Wo[:, g_cols] @ O_g -> [512, 4096] bf16. Host sums the 4
partials per batch and adds the residual + bo.

Device-side pipeline (per core):
  - xf (x[b] as [512, 4096]) fp8e4, packed so each [128, 1024] tile
    holds a contraction-chunk pair -> Q/K projections run as fp8
    DoubleRow matmuls (2 c-chunks per PE instruction), V^T as fp8+FWL.
  - Q, K: [128(gs), 4096] bf16; V^T: [128(keys-chunk), 32*128] fp8e4.
  - per query tile (512 q) and key group (256 k): S^T = K^T Q (bf16)
    as four N=256 matmuls into TWO psum tiles split by query half, so
    the two exp engines read disjoint tiles and run concurrently
    (sharing one tile makes the framework chain the readers): ScalarE
    exact exp for the low query half, a custom fused DVE op for the
    high half that writes e4m3 *bits* directly (bits = clamp(S*A + B8,
    0, 119), i.e. Schraudolph in log2 domain -- e4m3 bits are linear
    in log2 of the value). Then O += V^T E and denom += ones^T E as
    fp8 DoubleRow matmuls, two per group (one per query half).
  - normalize via reciprocal_approx_fast + ones-broadcast matmul, Wo
    partial matmuls, outputs stored bf16 via GpSimd copies.

The exp shift (-1) keeps E <= e^5 ~ 148 < 240 (e4m3 max) and cancels
in the softmax normalization.
"""

import os
import numpy as np
import ml_dtypes

B, C, N, G = 2, 512, 4096, 4
GS = C // G          # 128 head dim
SCALE = GS ** -0.5
ESHIFT = -1.0        # exp range shift, cancels in softmax
QT = 512             # query tile width
NQT = N // QT        # 8 query tiles
NKC = N // 128       # 32 key chunks
NCC = C // 128       # 4 contraction chunks for projections
NCP = NCC // 2       # 2 contraction chunk-pairs (DoubleRow)
NMC = C // 128       # 4 output-channel chunks

# Schraudolph exp -> e4m3 bits: bits = S*EA + EB, value 2^(bits/8 - 7).
# EA = 8*log2(e)*SCALE; EB = 8*log2(e)*ESHIFT + 56 + corr. corr centers
# the mantissa-linear decode error (-0.34) and the truncating f32->int8
# convert (+0.5).
EA = 8 * 1.4426950408889634 * SCALE
EB = 8 * 1.4426950408889634 * ESHIFT + 56.0 - 0.34 + 0.5
ECLAMP = 119.0       # max finite e4m3 bit pattern (240.0)

_compiled_nc = None
LAST_RESULT = None


def _register_exp8():
    """Register the fused DVE op out_i8 = max(min(in*s0 + s1, imm2), 0)
    via the documented dve_ops.OPS extension point."""
    import numpy as np
    import concourse.dve_ops as dve_ops
    from concourse.dve_spec import Spec, Src0, C0, C1, C2, Zero, maxx, minn

    for o in dve_ops.OPS:
        if o.name == "EXP8_SCHRAU":
            return o
    spec = Spec(
        body=maxx(minn(Src0 * C0 + C1, C2), Zero),
        reference=lambda in0, in1, c0, c1, c2:
            np.minimum(np.maximum(in0 * c0 + c1, 0.0), c2),
    )
    op = dve_ops.DveOp(
        "EXP8_SCHRAU", spec, subdim=False,
        uops_sha={"v3": "29868d14c5f51d0d", "v4": "ae82907ada109595"})
    dve_ops.OPS.append(op)
    dve_ops.CUSTOM_DVE_SPECS[op.name] = op.spec
    dve_ops._SUB_OPCODE_FOR_NAME[op.name] = (
        max(dve_ops._SUB_OPCODE_FOR_NAME.values()) + 1)
    return op


def _build():
    from contextlib import ExitStack
    import concourse.mybir as mybir
    import concourse.tile as tile
    import concourse.bass as cbass
    from concourse import bacc

    dt = mybir.dt
    f32 = dt.float32
    bf16 = dt.bfloat16
    fp8 = dt.float8e4
    i8 = dt.int8
    Exp = mybir.ActivationFunctionType.Exp
    DR = mybir.MatmulPerfMode.DoubleRow
    EXP8 = _register_exp8()

    nc = bacc.Bacc("TRN2", target_bir_lowering=False, debug=False, num_devices=8)

    # xb8: packed fp8 xf. Tile (nt, ccp) at cols (nt*NCP+ccp)*1024; within
    # a tile, c-chunk 2*ccp at cols 0:512 and 2*ccp+1 at 512:1024.
    xb8 = nc.dram_tensor("xb8", [128, N * NCC], fp8, kind="ExternalInput")
    # packed weights: wX_p[:, cc*128:(cc+1)*128] is the [128(c), 128(gs)]
    # lhsT chunk for contraction chunk cc.
    wq_p = nc.dram_tensor("wq_p", [128, C], fp8, kind="ExternalInput")
    wk_p = nc.dram_tensor("wk_p", [128, C], fp8, kind="ExternalInput")
    wv_p = nc.dram_tensor("wv_p", [128, C], fp8, kind="ExternalInput")
    woT = nc.dram_tensor("woT", [GS, C], bf16, kind="ExternalInput")
    # bias pack: col 0 = bq, col 1 = bk, cols 2..129 = bv broadcast rows
    bias_p = nc.dram_tensor("bias_p", [128, 130], f32, kind="ExternalInput")
    outp = nc.dram_tensor("outp", [C, N], bf16, kind="ExternalOutput")

    KG = 2                    # key chunks per exp group
    GW = KG * 128             # exp group width in keys
    NGR = N // GW             # 16 groups per query tile

    with tile.TileContext(nc) as tc, ExitStack() as ctx:
        persist = ctx.enter_context(tc.tile_pool(name="persist", bufs=1))
        epool = ctx.enter_context(tc.tile_pool(name="epool", bufs=6))
        spool = ctx.enter_context(tc.tile_pool(name="spool", bufs=6))
        # PSUM budget (8 banks): psS 2x[128,1024]=4, psO 2x[128,512]=2,
        # psD 1, psP 1.
        psS = ctx.enter_context(tc.tile_pool(name="psS", bufs=2, space="PSUM"))
        psO = ctx.enter_context(tc.tile_pool(name="psO", bufs=2, space="PSUM"))
        psD = ctx.enter_context(tc.tile_pool(name="psD", bufs=1, space="PSUM"))
        psP = ctx.enter_context(tc.tile_pool(name="psP", bufs=1, space="PSUM"))

        def load(shape, dtype, dram_ap, tag, eng=None):
            t = persist.tile(shape, dtype, tag=tag)
            (eng or nc.sync).dma_start(t[:], dram_ap)
            return t

        # Weights: 5 packed DMAs on the sync queue, in first-use order.
        wq_all = load([128, C], fp8, wq_p[:, :], "wq")
        wk_all = load([128, C], fp8, wk_p[:, :], "wk")
        wv_all = load([128, C], fp8, wv_p[:, :], "wv")
        bias_sb = load([128, 130], f32, bias_p[:, :], "bias")
        wo_sb = load([GS, C], bf16, woT[:, :], "wo")
        wv = [wv_all[:, cc * 128:(cc + 1) * 128] for cc in range(NCC)]
        bq_sb = bias_sb[:, 0:1]
        bk_sb = bias_sb[:, 1:2]
        bvb_sb = bias_sb[:, 2:130]

        def wpair(w_all, cp):
            return w_all[:, cp * 256:(cp + 1) * 256].rearrange(
                "p (two m) -> p two m", two=2)

        xf2 = [[None] * NQT for _ in range(NCP)]

        def load_xf(nt):
            # first two nt's alternate scalar/gpsimd so the earliest tiles
            # land in parallel; the rest go on gpsimd (sync carries the
            # weight DMAs + V^T transposes; scalar triggers would block
            # the bias adds)
            for cp in range(NCP):
                idx = nt * NCP + cp
                if idx < 4:
                    eng = nc.scalar if idx % 2 == 0 else nc.gpsimd
                else:
                    eng = nc.gpsimd
                col0 = idx * 1024
                xf2[cp][nt] = load(
                    [128, 1024], fp8, xb8[:, col0:col0 + 1024],
                    f"xf{cp}_{nt}", eng=eng)

        # fp8 ones for the DoubleRow denominator matmul: view with a
        # 16-byte stride on the pair dim to satisfy the DR weight AP rule.
        ones_k8 = persist.tile([128, 32], fp8, tag="ones_k8")
        nc.vector.memset(ones_k8[:], 1.0)
        ones2 = ones_k8[:, 0:17:16].rearrange("p (two m) -> p two m", two=2)
        ones_1 = persist.tile([1, 128], bf16, tag="ones_1")
        nc.vector.memset(ones_1[:], 1.0)
        eshift = persist.tile([128, 1], f32, tag="eshift")
        nc.vector.memset(eshift[:], ESHIFT)

        q_sb = persist.tile([GS, N], bf16, tag="q_sb")
        k_sb = persist.tile([GS, N], bf16, tag="k_sb")
        vt_sb = persist.tile([128, N], fp8, tag="vt_sb")

        # Projections, emitted per column block (nt) with its xf loads
        # inline so compute starts after the first 2 DMAs. Q/K/V all run
        # as fp8 DoubleRow over c-chunk pairs (stream-bound, ~1us/nt).
        # V comes out [gs, keys]; the DMA transpose XBAR flips each
        # 128-key chunk to V^T off the PE, then ScalarE casts to fp8.
        load_xf(0)
        for nt in range(NQT):
            if nt + 1 < NQT:
                load_xf(nt + 1)
            nsl = slice(nt * QT, (nt + 1) * QT)
            for w_all, b_t, dst in ((wq_all, bq_sb, q_sb),
                                    (wk_all, bk_sb, k_sb)):
                ps = psO.tile([128, QT], f32, tag="po")
                for cp in range(NCP):
                    nc.tensor.matmul(
                        ps[:], wpair(w_all, cp),
                        xf2[cp][nt][:].rearrange("p (two m) -> p two m", two=2),
                        perf_mode=DR, start=(cp == 0), stop=(cp == NCP - 1))
                nc.scalar.add(dst[:, nsl], ps[:], b_t)
            for kc in range(nt * QT // 128, (nt + 1) * QT // 128):
                ksl = slice(kc * 128, (kc + 1) * 128)
                off = kc * 128 - nt * QT
                ps = psS.tile([128, GS], f32, tag=f"ps{kc % 2}")
                for cc in range(NCC):
                    cp, j = divmod(cc, 2)
                    xsl = xf2[cp][nt][:, j * 512 + off:j * 512 + off + 128]
                    nc.tensor.matmul(ps[:], xsl, wv[cc],
                                     start=(cc == 0), stop=(cc == NCC - 1))
                nc.vector.tensor_add(vt_sb[:, ksl], ps[:], bvb_sb)

        # Attention, software-pipelined over a flat (qt, g) stream so the
        # S matmuls cross query-tile boundaries with no pipeline drain.
        # S for one (qt, group) is computed into TWO psum tiles split by
        # query half, so the two exp engines read disjoint tiles and the
        # framework can't chain them behind each other.
        QH = QT // 2

        def emit_S(qt, g):
            ps = []
            for h in range(2):
                qsl = slice(qt * QT + h * QH, qt * QT + (h + 1) * QH)
                p = psS.tile([128, GW // 128 * QH], f32, tag=f"ps{h}")
                for j in range(KG):
                    kc = g * KG + j
                    ksl = slice(kc * 128, (kc + 1) * 128)
                    nc.tensor.matmul(p[:, j * QH:(j + 1) * QH],
                                     k_sb[:, ksl], q_sb[:, qsl],
                                     start=True, stop=True)
                ps.append(p)
            return ps

        def emit_tail(qt, po, pd, last=False):
            state = {}

            def tail_pre():
                # free the pd bank + start the reciprocal chain early:
                # reciprocal on the [1,512] denominator row, then a
                # stride-0-partition gpsimd DMA broadcasts it to 128
                # partitions -- no PE broadcast matmul needed
                den_f = spool.tile([1, QT], f32, tag="denf")
                nc.vector.tensor_copy(den_f[:], pd[:])
                binv1 = spool.tile([1, QT], f32, tag="binv1")
                nc.vector.reciprocal_approx_fast(binv1[:], den_f[:])
                binv = spool.tile([128, QT], f32, tag="binv")
                src = binv1[:]
                bsrc = cbass.AP(
                    tensor=src.tensor, offset=src.offset,
                    ap=[list(src.ap[0]), [0, 128]] + list(src.ap[1:]))
                nc.gpsimd.dma_start(out=binv[:], in_=bsrc)
                state["binv"] = binv

            def tail_mc(mc):
                # one Wo output chunk; called at successive groups so the
                # psum-bank serialization and the copies spread out
                # instead of stalling one group's exps
                qsl = slice(qt * QT, (qt + 1) * QT)
                if mc == 0:
                    o_sb = spool.tile([128, QT], bf16, tag="osb")
                    nc.vector.tensor_mul(o_sb[:], po[:], state["binv"][:])
                    state["osb"] = o_sb
                msl = slice(mc * 128, (mc + 1) * 128)
                pool = psS if last else psP
                pp = pool.tile([128, QT], f32,
                               tag=f"ps{mc % 2}" if last else "pp")
                nc.tensor.matmul(pp[:], wo_sb[:, msl], state["osb"][:],
                                 start=True, stop=True)
                st = spool.tile([128, QT], bf16, tag="st")
                if last and mc % 2 == 0:
                    nc.vector.tensor_copy(st[:], pp[:])
                else:
                    nc.scalar.copy(st[:], pp[:])
                st_eng = nc.gpsimd if (last and mc % 2 == 0) else nc.sync
                st_eng.dma_start(outp[msl, qsl], st[:])
            return tail_pre, tail_mc

        tails = []
        pairs = [(qt, g) for qt in range(NQT) for g in range(NGR)]
        s_cur = emit_S(0, 0)
        po = pd = None
        elo_g, ehi_g = [], []
        for i, (qt, g) in enumerate(pairs):
            if g == 0:
                if tails:
                    tails[-1][0]()   # prev epilogue: den copy + bcast + recip
                po = psO.tile([128, QT], f32, tag="po")
                pd = psD.tile([1, QT], f32, tag="pd")
            s_next = emit_S(*pairs[i + 1]) if i + 1 < len(pairs) else None
            # rest of prev epilogue spread over groups 3/5/7/9: the DVE
            # reciprocal chain is done by g==3, and spacing the Wo chunks
            # avoids bursting 4 copies into one group's engine FIFOs
            if g in (3, 5, 7, 9) and tails:
                tails[-1][1]((g - 3) // 2)
                if g == 9:
                    tails.pop()
            # exp split by query half across both engines: each engine
            # reads its own psum tile and writes its own fp8 tile, so
            # they run fully concurrently.
            e_lo = epool.tile([128, 2 * QH], fp8, tag="elo")
            e_hi = epool.tile([128, 2 * QH], fp8, tag="ehi")
            lo3 = e_lo[:].rearrange("p (two m) -> p two m", two=2)
            hi3 = e_hi[:].rearrange("p (two m) -> p two m", two=2)
            nc.scalar.activation(e_lo[:], s_cur[0][:], Exp,
                                 scale=SCALE, bias=eshift[:])
            nc.vector._custom_dve(
                EXP8, out=e_hi[:].bitcast(i8), in0=s_cur[1][:],
                s0=EA, s1=EB, imm2=ECLAMP)
            vt3 = vt_sb[:, g * GW:(g + 1) * GW].rearrange(
                "p (two m) -> p two m", two=2)
            st_fl = dict(start=(g == 0), stop=(g == NGR - 1))
            nc.tensor.matmul(po[:, :QH], vt3, lo3, perf_mode=DR, **st_fl)
            nc.tensor.matmul(po[:, QH:], vt3, hi3, perf_mode=DR, **st_fl)
            nc.tensor.matmul(pd[:, :QH], ones2, lo3, perf_mode=DR, **st_fl)
            nc.tensor.matmul(pd[:, QH:], ones2, hi3, perf_mode=DR, **st_fl)
            s_cur = s_next
            if g == NGR - 1:
                tails.append(emit_tail(qt, po, pd, last=(qt == NQT - 1)))
        tp, tm = tails.pop()
        tp()
        for mc in range(NMC):
            tm(mc)

    nc.compile()
    return nc


def _get_compiled():
    global _compiled_nc
    if _compiled_nc is None:
        _compiled_nc = _build()
    return _compiled_nc


def _ensure_ntff_hook():
    """Best-effort: register the axon NTFF profile hook so trace=True
    yields exec_time_ns. The image's antenv lacks axon_hooks; shim it."""
    import sys, types
    try:
        from antenv.axon_hooks import get_axon_ntff_profile_hook  # noqa: F401
        return
    except ImportError:
        pass
    try:
        mod = types.ModuleType("antenv.axon_hooks")
        _hook = [None]
        mod.set_axon_ntff_profile_hook = lambda h: _hook.__setitem__(0, h)
        mod.get_axon_ntff_profile_hook = lambda: _hook[0]
        sys.modules["antenv.axon_hooks"] = mod
        import antenv
        antenv.axon_hooks = mod
        from trn_agent_boot.trn_boot import _ntff_profile_via_ctypes
        mod.set_axon_ntff_profile_hook(
            _ntff_profile_via_ctypes("/opt/axon/libaxon_pjrt.so"))
    except Exception:
        pass


def kernel(x, Wq, bq, Wk, bk, Wv, bv, Wo, bo):
    global LAST_RESULT
    from concourse.bass_utils import run_bass_kernel_spmd

    nc = _get_compiled()
    bf = ml_dtypes.bfloat16
    f8 = ml_dtypes.float8_e4m3
    x = np.asarray(x, dtype=np.float32)
    b, c, d, h, w = x.shape
    n = d * h * w
    xf = x.reshape(b, c, n)
    Wq = np.asarray(Wq, np.float32)
    Wk = np.asarray(Wk, np.float32)
    Wv = np.asarray(Wv, np.float32)
    Wo = np.asarray(Wo, np.float32)
    bq = np.asarray(bq, np.float32)
    bk = np.asarray(bk, np.float32)
    bv = np.asarray(bv, np.float32)
    bo = np.asarray(bo, np.float32)

    def pack_w(WT):
        # [512(c), 128(gs)] -> [128, 512] with chunk cc at cols cc*128:
        return np.ascontiguousarray(
            WT.reshape(4, 128, 128).transpose(1, 0, 2).reshape(128, 512))

    def pack_x(xb):
        # [512(c), 4096] -> [128, 16384]: tile (nt, ccp) at cols
        # (nt*2+ccp)*1024, c-chunk 2ccp+j within it at cols j*512.
        t = xb.reshape(2, 2, 128, 8, 512)        # [ccp, j, p, nt, m]
        t = t.transpose(2, 3, 0, 1, 4)           # [p, nt, ccp, j, m]
        return np.ascontiguousarray(t.reshape(128, 16384))

    in_maps = []
    for core in range(8):
        bb, g = divmod(core, G)
        gsl = slice(g * GS, (g + 1) * GS)
        bias_p = np.empty((128, 130), np.float32)
        bias_p[:, 0] = bq[gsl]
        bias_p[:, 1] = bk[gsl]
        bias_p[:, 2:130] = np.broadcast_to(bv[gsl], (128, GS))
        in_maps.append({
            "xb8": pack_x(xf[bb]).astype(f8),
            "wq_p": pack_w(Wq[gsl, :].T).astype(f8),
            "wk_p": pack_w(Wk[gsl, :].T).astype(f8),
            "wv_p": pack_w(Wv[gsl, :].T).astype(f8),
            "woT": np.ascontiguousarray(Wo[:, gsl].T).astype(bf),
            "bias_p": bias_p,
        })

    trace = bool(os.environ.get("BASS_TRACE"))
    if trace:
        _ensure_ntff_hook()
    LAST_RESULT = run_bass_kernel_spmd(
        nc, in_maps, core_ids=list(range(8)), trace=trace)
    outs = LAST_RESULT.results

    out = np.empty((b, c, n), np.float32)
    for bb in range(b):
        acc = xf[bb] + bo[:, None]
        for g in range(G):
            acc = acc + outs[bb * G + g]["outp"].astype(np.float32)
        out[bb] = acc
    return out.reshape(b, c, d, h, w)


# revision 72
# speedup vs baseline: 1.4231x; 1.4231x over previous
"""Grouped-query attention kernel for 8 Trainium2 NeuronCores.

Problem (hardcoded): x [2, 512, 16, 16, 16] f32, Wq/Wk/Wv/Wo [512, 512],
biases [512]. G=4 heads of dim 128, N=4096 tokens. out = x + Wo@attn.

Sharding: one (batch, group) pair per core -> 8 cores, no cross-core
communication. Each core computes its group's Q/K/V projections, the
full 4096x4096 attention for its (b, g), and a partial output
projection Wo[:, g_cols] @ O_g -> [512, 4096] bf16. Host sums the 4
partials per batch and adds the residual + bo.

Device-side pipeline (per core):
  - xf (x[b] as [512, 4096]) fp8e4, packed so each [128, 1024] tile
    holds a contraction-chunk pair -> Q/K projections run as fp8
    DoubleRow matmuls (2 c-chunks per PE instruction), V^T as fp8+FWL.
  - Q, K: [128(gs), 4096] bf16; V^T: [128(keys-chunk), 32*128] fp8e4.
  - per query tile (512 q) and key group (256 k): S^T = K^T Q (bf16)
    as four N=256 matmuls into TWO psum tiles split by query half, so
    the two exp engines read disjoint tiles and run concurrently
    (sharing one tile makes the framework chain the readers): ScalarE
    exact exp for the low query half, a custom fused DVE op for the
    high half that writes e4m3 *bits* directly (bits = clamp(S*A + B8,
    0, 119), i.e. Schraudolph in log2 domain -- e4m3 bits are linear
    in log2 of the value). Then O += V^T E and denom += ones^T E as
    fp8 DoubleRow matmuls, two per group (one per query half).
  - normalize via reciprocal_approx_fast + ones-broadcast matmul, Wo
    partial matmuls, outputs stored bf16 via GpSimd copies.

The exp shift (-1) keeps E <= e^5 ~ 148 < 240 (e4m3 max) and cancels
in the softmax normalization.
"""

import os
import numpy as np
import ml_dtypes

B, C, N, G = 2, 512, 4096, 4
GS = C // G          # 128 head dim
SCALE = GS ** -0.5
ESHIFT = -1.0        # exp range shift, cancels in softmax
QT = 512             # query tile width
NQT = N // QT        # 8 query tiles
NKC = N // 128       # 32 key chunks
NCC = C // 128       # 4 contraction chunks for projections
NCP = NCC // 2       # 2 contraction chunk-pairs (DoubleRow)
NMC = C // 128       # 4 output-channel chunks

# Schraudolph exp -> e4m3 bits: bits = S*EA + EB, value 2^(bits/8 - 7).
# EA = 8*log2(e)*SCALE; EB = 8*log2(e)*ESHIFT + 56 + corr. corr centers
# the mantissa-linear decode error (-0.34) and the truncating f32->int8
# convert (+0.5).
EA = 8 * 1.4426950408889634 * SCALE
EB = 8 * 1.4426950408889634 * ESHIFT + 56.0 - 0.34 + 0.5
ECLAMP = 119.0       # max finite e4m3 bit pattern (240.0)

_compiled_nc = None
LAST_RESULT = None


def _register_exp8():
    """Register the fused DVE op out_i8 = max(min(in*s0 + s1, imm2), 0)
    via the documented dve_ops.OPS extension point."""
    import numpy as np
    import concourse.dve_ops as dve_ops
    from concourse.dve_spec import Spec, Src0, C0, C1, C2, Zero, maxx, minn

    for o in dve_ops.OPS:
        if o.name == "EXP8_SCHRAU":
            return o
    spec = Spec(
        body=maxx(minn(Src0 * C0 + C1, C2), Zero),
        reference=lambda in0, in1, c0, c1, c2:
            np.minimum(np.maximum(in0 * c0 + c1, 0.0), c2),
    )
    op = dve_ops.DveOp(
        "EXP8_SCHRAU", spec, subdim=False,
        uops_sha={"v3": "29868d14c5f51d0d", "v4": "ae82907ada109595"})
    dve_ops.OPS.append(op)
    dve_ops.CUSTOM_DVE_SPECS[op.name] = op.spec
    dve_ops._SUB_OPCODE_FOR_NAME[op.name] = (
        max(dve_ops._SUB_OPCODE_FOR_NAME.values()) + 1)
    return op


def _build():
    from contextlib import ExitStack
    import concourse.mybir as mybir
    import concourse.tile as tile
    import concourse.bass as cbass
    from concourse import bacc

    dt = mybir.dt
    f32 = dt.float32
    bf16 = dt.bfloat16
    fp8 = dt.float8e4
    i8 = dt.int8
    Exp = mybir.ActivationFunctionType.Exp
    DR = mybir.MatmulPerfMode.DoubleRow
    EXP8 = _register_exp8()

    nc = bacc.Bacc("TRN2", target_bir_lowering=False, debug=False, num_devices=8)

    # xb8: packed fp8 xf. Tile (nt, ccp) at cols (nt*NCP+ccp)*1024; within
    # a tile, c-chunk 2*ccp at cols 0:512 and 2*ccp+1 at 512:1024.
    xb8 = nc.dram_tensor("xb8", [128, N * NCC], fp8, kind="ExternalInput")
    # packed weights: wX_p[:, cc*128:(cc+1)*128] is the [128(c), 128(gs)]
    # lhsT chunk for contraction chunk cc.
    wq_p = nc.dram_tensor("wq_p", [128, C], fp8, kind="ExternalInput")
    wk_p = nc.dram_tensor("wk_p", [128, C], fp8, kind="ExternalInput")
    wv_p = nc.dram_tensor("wv_p", [128, C], fp8, kind="ExternalInput")
    woT = nc.dram_tensor("woT", [GS, C], bf16, kind="ExternalInput")
    # bias pack: col 0 = bq, col 1 = bk, cols 2..129 = bv broadcast rows
    bias_p = nc.dram_tensor("bias_p", [128, 130], f32, kind="ExternalInput")
    outp = nc.dram_tensor("outp", [C, N], bf16, kind="ExternalOutput")

    KG = 2                    # key chunks per exp group
    GW = KG * 128             # exp group width in keys
    NGR = N // GW             # 16 groups per query tile

    with tile.TileContext(nc) as tc, ExitStack() as ctx:
        persist = ctx.enter_context(tc.tile_pool(name="persist", bufs=1))
        epool = ctx.enter_context(tc.tile_pool(name="epool", bufs=6))
        spool = ctx.enter_context(tc.tile_pool(name="spool", bufs=6))
        # PSUM budget (8 banks): psS 2x[128,1024]=4, psO 2x[128,512]=2,
        # psD 1, psP 1.
        psS = ctx.enter_context(tc.tile_pool(name="psS", bufs=2, space="PSUM"))
        psO = ctx.enter_context(tc.tile_pool(name="psO", bufs=2, space="PSUM"))
        psD = ctx.enter_context(tc.tile_pool(name="psD", bufs=1, space="PSUM"))
        psP = ctx.enter_context(tc.tile_pool(name="psP", bufs=1, space="PSUM"))

        def load(shape, dtype, dram_ap, tag, eng=None):
            t = persist.tile(shape, dtype, tag=tag)
            (eng or nc.sync).dma_start(t[:], dram_ap)
            return t

        # Weights: 5 packed DMAs on the sync queue, in first-use order.
        wq_all = load([128, C], fp8, wq_p[:, :], "wq")
        wk_all = load([128, C], fp8, wk_p[:, :], "wk")
        wv_all = load([128, C], fp8, wv_p[:, :], "wv")
        bias_sb = load([128, 130], f32, bias_p[:, :], "bias")
        wo_sb = load([GS, C], bf16, woT[:, :], "wo")
        wv = [wv_all[:, cc * 128:(cc + 1) * 128] for cc in range(NCC)]
        bq_sb = bias_sb[:, 0:1]
        bk_sb = bias_sb[:, 1:2]
        bvb_sb = bias_sb[:, 2:130]

        def wpair(w_all, cp):
            return w_all[:, cp * 256:(cp + 1) * 256].rearrange(
                "p (two m) -> p two m", two=2)

        xf2 = [[None] * NQT for _ in range(NCP)]

        def load_xf(nt):
            # first two nt's alternate scalar/gpsimd so the earliest tiles
            # land in parallel; the rest go on gpsimd (sync carries the
            # weight DMAs + V^T transposes; scalar triggers would block
            # the bias adds)
            for cp in range(NCP):
                idx = nt * NCP + cp
                if idx < 4:
                    eng = nc.scalar if idx % 2 == 0 else nc.gpsimd
                else:
                    eng = nc.gpsimd
                col0 = idx * 1024
                xf2[cp][nt] = load(
                    [128, 1024], fp8, xb8[:, col0:col0 + 1024],
                    f"xf{cp}_{nt}", eng=eng)

        # fp8 ones for the DoubleRow denominator matmul: view with a
        # 16-byte stride on the pair dim to satisfy the DR weight AP rule.
        ones_k8 = persist.tile([128, 32], fp8, tag="ones_k8")
        nc.vector.memset(ones_k8[:], 1.0)
        ones2 = ones_k8[:, 0:17:16].rearrange("p (two m) -> p two m", two=2)
        ones_1 = persist.tile([1, 128], bf16, tag="ones_1")
        nc.vector.memset(ones_1[:], 1.0)
        eshift = persist.tile([128, 1], f32, tag="eshift")
        nc.vector.memset(eshift[:], ESHIFT)

        q_sb = persist.tile([GS, N], bf16, tag="q_sb")
        k_sb = persist.tile([GS, N], bf16, tag="k_sb")
        vt_sb = persist.tile([128, N], fp8, tag="vt_sb")

        # Projections, emitted per column block (nt) with its xf loads
        # inline so compute starts after the first 2 DMAs. Q/K/V all run
        # as fp8 DoubleRow over c-chunk pairs (stream-bound, ~1us/nt).
        # V comes out [gs, keys]; the DMA transpose XBAR flips each
        # 128-key chunk to V^T off the PE, then ScalarE casts to fp8.
        load_xf(0)
        for nt in range(NQT):
            if nt + 1 < NQT:
                load_xf(nt + 1)
            nsl = slice(nt * QT, (nt + 1) * QT)
            for w_all, b_t, dst in ((wq_all, bq_sb, q_sb),
                                    (wk_all, bk_sb, k_sb)):
                ps = psO.tile([128, QT], f32, tag="po")
                for cp in range(NCP):
                    nc.tensor.matmul(
                        ps[:], wpair(w_all, cp),
                        xf2[cp][nt][:].rearrange("p (two m) -> p two m", two=2),
                        perf_mode=DR, start=(cp == 0), stop=(cp == NCP - 1))
                nc.scalar.add(dst[:, nsl], ps[:], b_t)
            for kc in range(nt * QT // 128, (nt + 1) * QT // 128):
                ksl = slice(kc * 128, (kc + 1) * 128)
                off = kc * 128 - nt * QT
                ps = psS.tile([128, GS], f32, tag=f"ps{kc % 2}")
                for cc in range(NCC):
                    cp, j = divmod(cc, 2)
                    xsl = xf2[cp][nt][:, j * 512 + off:j * 512 + off + 128]
                    nc.tensor.matmul(ps[:], xsl, wv[cc],
                                     start=(cc == 0), stop=(cc == NCC - 1))
                nc.vector.tensor_add(vt_sb[:, ksl], ps[:], bvb_sb)

        # Attention, software-pipelined over a flat (qt, g) stream so the
        # S matmuls cross query-tile boundaries with no pipeline drain.
        # S for one (qt, group) is computed into TWO psum tiles split by
        # query half, so the two exp engines read disjoint tiles and the
        # framework can't chain them behind each other.
        QH = QT // 2

        def emit_S(qt, g):
            ps = []
            for h in range(2):
                qsl = slice(qt * QT + h * QH, qt * QT + (h + 1) * QH)
                p = psS.tile([128, GW // 128 * QH], f32, tag=f"ps{h}")
                for j in range(KG):
                    kc = g * KG + j
                    ksl = slice(kc * 128, (kc + 1) * 128)
                    nc.tensor.matmul(p[:, j * QH:(j + 1) * QH],
                                     k_sb[:, ksl], q_sb[:, qsl],
                                     start=True, stop=True)
                ps.append(p)
            return ps

        def emit_tail(qt, po, pd, last=False):
            state = {}

            def tail_pre():
                # free the pd bank + start the reciprocal chain early
                den_sb = spool.tile([1, QT], bf16, tag="den")
                nc.vector.tensor_copy(den_sb[:], pd[:])
                pb = psP.tile([128, QT], f32, tag="pp")
                nc.tensor.matmul(pb[:], ones_1[:], den_sb[:],
                                 start=True, stop=True)
                binv = spool.tile([128, QT], f32, tag="binv")
                nc.vector.reciprocal_approx_fast(binv[:], pb[:])
                state["binv"] = binv

            def tail_mc(mc):
                # one Wo output chunk; called at successive groups so the
                # psum-bank serialization and the copies spread out
                # instead of stalling one group's exps
                qsl = slice(qt * QT, (qt + 1) * QT)
                if mc == 0:
                    o_sb = spool.tile([128, QT], bf16, tag="osb")
                    nc.vector.tensor_mul(o_sb[:], po[:], state["binv"][:])
                    state["osb"] = o_sb
                msl = slice(mc * 128, (mc + 1) * 128)
                pool = psS if last else psP
                pp = pool.tile([128, QT], f32,
                               tag=f"ps{mc % 2}" if last else "pp")
                nc.tensor.matmul(pp[:], wo_sb[:, msl], state["osb"][:],
                                 start=True, stop=True)
                st = spool.tile([128, QT], bf16, tag="st")
                if last and mc % 2 == 0:
                    nc.vector.tensor_copy(st[:], pp[:])
                else:
                    nc.scalar.copy(st[:], pp[:])
                st_eng = nc.gpsimd if (last and mc % 2 == 0) else nc.sync
                st_eng.dma_start(outp[msl, qsl], st[:])
            return tail_pre, tail_mc

        tails = []
        pairs = [(qt, g) for qt in range(NQT) for g in range(NGR)]
        s_cur = emit_S(0, 0)
        po = pd = None
        elo_g, ehi_g = [], []
        for i, (qt, g) in enumerate(pairs):
            if g == 0:
                if tails:
                    tails[-1][0]()   # prev epilogue: den copy + bcast + recip
                po = psO.tile([128, QT], f32, tag="po")
                pd = psD.tile([1, QT], f32, tag="pd")
            s_next = emit_S(*pairs[i + 1]) if i + 1 < len(pairs) else None
            # rest of prev epilogue spread over groups 3/5/7/9: the DVE
            # reciprocal chain is done by g==3, and spacing the Wo chunks
            # avoids bursting 4 copies into one group's engine FIFOs
            if g in (3, 5, 7, 9) and tails:
                tails[-1][1]((g - 3) // 2)
                if g == 9:
                    tails.pop()
            # exp split by query half across both engines: each engine
            # reads its own psum tile and writes its own fp8 tile, so
            # they run fully concurrently.
            e_lo = epool.tile([128, 2 * QH], fp8, tag="elo")
            e_hi = epool.tile([128, 2 * QH], fp8, tag="ehi")
            lo3 = e_lo[:].rearrange("p (two m) -> p two m", two=2)
            hi3 = e_hi[:].rearrange("p (two m) -> p two m", two=2)
            nc.scalar.activation(e_lo[:], s_cur[0][:], Exp,
                                 scale=SCALE, bias=eshift[:])
            nc.vector._custom_dve(
                EXP8, out=e_hi[:].bitcast(i8), in0=s_cur[1][:],
                s0=EA, s1=EB, imm2=ECLAMP)
            vt3 = vt_sb[:, g * GW:(g + 1) * GW].rearrange(
                "p (two m) -> p two m", two=2)
            st_fl = dict(start=(g == 0), stop=(g == NGR - 1))
            nc.tensor.matmul(po[:, :QH], vt3, lo3, perf_mode=DR, **st_fl)
            nc.tensor.matmul(po[:, QH:], vt3, hi3, perf_mode=DR, **st_fl)
            nc.tensor.matmul(pd[:, :QH], ones2, lo3, perf_mode=DR, **st_fl)
            nc.tensor.matmul(pd[:, QH:], ones2, hi3, perf_mode=DR, **st_fl)
            s_cur = s_next
            if g == NGR - 1:
                tails.append(emit_tail(qt, po, pd, last=(qt == NQT - 1)))
        tp, tm = tails.pop()
        tp()
        for mc in range(NMC):
            tm(mc)

    nc.compile()
    return nc


def _get_compiled():
    global _compiled_nc
    if _compiled_nc is None:
        _compiled_nc = _build()
    return _compiled_nc


def _ensure_ntff_hook():
    """Best-effort: register the axon NTFF profile hook so trace=True
    yields exec_time_ns. The image's antenv lacks axon_hooks; shim it."""
    import sys, types
    try:
        from antenv.axon_hooks import get_axon_ntff_profile_hook  # noqa: F401
        return
    except ImportError:
        pass
    try:
        mod = types.ModuleType("antenv.axon_hooks")
        _hook = [None]
        mod.set_axon_ntff_profile_hook = lambda h: _hook.__setitem__(0, h)
        mod.get_axon_ntff_profile_hook = lambda: _hook[0]
        sys.modules["antenv.axon_hooks"] = mod
        import antenv
        antenv.axon_hooks = mod
        from trn_agent_boot.trn_boot import _ntff_profile_via_ctypes
        mod.set_axon_ntff_profile_hook(
            _ntff_profile_via_ctypes("/opt/axon/libaxon_pjrt.so"))
    except Exception:
        pass


def kernel(x, Wq, bq, Wk, bk, Wv, bv, Wo, bo):
    global LAST_RESULT
    from concourse.bass_utils import run_bass_kernel_spmd

    nc = _get_compiled()
    bf = ml_dtypes.bfloat16
    f8 = ml_dtypes.float8_e4m3
    x = np.asarray(x, dtype=np.float32)
    b, c, d, h, w = x.shape
    n = d * h * w
    xf = x.reshape(b, c, n)
    Wq = np.asarray(Wq, np.float32)
    Wk = np.asarray(Wk, np.float32)
    Wv = np.asarray(Wv, np.float32)
    Wo = np.asarray(Wo, np.float32)
    bq = np.asarray(bq, np.float32)
    bk = np.asarray(bk, np.float32)
    bv = np.asarray(bv, np.float32)
    bo = np.asarray(bo, np.float32)

    def pack_w(WT):
        # [512(c), 128(gs)] -> [128, 512] with chunk cc at cols cc*128:
        return np.ascontiguousarray(
            WT.reshape(4, 128, 128).transpose(1, 0, 2).reshape(128, 512))

    def pack_x(xb):
        # [512(c), 4096] -> [128, 16384]: tile (nt, ccp) at cols
        # (nt*2+ccp)*1024, c-chunk 2ccp+j within it at cols j*512.
        t = xb.reshape(2, 2, 128, 8, 512)        # [ccp, j, p, nt, m]
        t = t.transpose(2, 3, 0, 1, 4)           # [p, nt, ccp, j, m]
        return np.ascontiguousarray(t.reshape(128, 16384))

    in_maps = []
    for core in range(8):
        bb, g = divmod(core, G)
        gsl = slice(g * GS, (g + 1) * GS)
        bias_p = np.empty((128, 130), np.float32)
        bias_p[:, 0] = bq[gsl]
        bias_p[:, 1] = bk[gsl]
        bias_p[:, 2:130] = np.broadcast_to(bv[gsl], (128, GS))
        in_maps.append({
            "xb8": pack_x(xf[bb]).astype(f8),
            "wq_p": pack_w(Wq[gsl, :].T).astype(f8),
            "wk_p": pack_w(Wk[gsl, :].T).astype(f8),
            "wv_p": pack_w(Wv[gsl, :].T).astype(f8),
            "woT": np.ascontiguousarray(Wo[:, gsl].T).astype(bf),
            "bias_p": bias_p,
        })

    trace = bool(os.environ.get("BASS_TRACE"))
    if trace:
        _ensure_ntff_hook()
    LAST_RESULT = run_bass_kernel_spmd(
        nc, in_maps, core_ids=list(range(8)), trace=trace)
    outs = LAST_RESULT.results

    out = np.empty((b, c, n), np.float32)
    for bb in range(b):
        acc = xf[bb] + bo[:, None]
        for g in range(G):
            acc = acc + outs[bb * G + g]["outp"].astype(np.float32)
        out[bb] = acc
    return out.reshape(b, c, d, h, w)
